# revision 33
# baseline (speedup 1.0000x reference)
"""MoE MLP (top-2 routing, capacity 1.25) on 8 Trainium2 NeuronCores.

Strategy (expert-parallel, per the sharding hint):
  - Router + top-k + capacity assignment run on host in float64 (cheap:
    0.27 GFLOP vs 344 GFLOP for the expert FFNs, and data-dependent
    control flow is a poor fit for the static Bass dataflow graph).
  - Every expert overflows capacity for this problem size (mean load
    4096 assignments vs cap 2560), so each of the 8 cores computes a
    dense [cap,D] @ [D,F] -> gelu -> [cap,F] @ [F,D] FFN for one expert.
  - Dispatch/combine (gather/scatter by routing indices) run on host.

Device kernel layout: activations are kept feature-major ([D, cap] /
[F, cap]) so both matmuls use weight tiles as the stationary operand and
no transposes are needed anywhere.  Two variants (MODE below):
  - "f32r" (default): float32r matmuls (fp32 operands, ~tf32 internal
    precision, fp32 PSUM accumulate).  Both weight stacks can't stay
    resident in fp32 (33.6 MB > 24 MB usable SBUF), so weights stream
    from HBM every token tile in host-pre-packed linear blocks; ~190 MB
    of DMA against ~600us of PE time - right at the compute/memory
    ridge.  ~618us, rel err ~2.1e-4.
  - "bf16": weights cast to bf16 on host and kept resident in SBUF;
    minimal DMA, pure PE-bound.  ~576us, rel err ~3.4e-3.
"""

import numpy as np
import ml_dtypes

B, T, D, F_FF, E, TOP_K = 8, 2048, 1024, 4096, 8, 2
N = B * T
CAP = 2560          # int(1.25 * N / E)
NCORES = 8
P = 128
DC = D // P         # 8 chunks of the model dim
FC = F_FF // P      # 32 chunks of the ff dim
NT = 512            # token tile (one PSUM bank of fp32)
TT = CAP // NT      # 5 token tiles

BF16 = ml_dtypes.bfloat16

# "f32r": weights streamed, float32r matmuls (~618us, rel err ~2.1e-4)
# "bf16": weights resident in SBUF, bf16 matmuls (~576us, rel err ~3.4e-3)
MODE = "f32r"

_NC_CACHE = {}


def _build_nc():
    """Per-core Bass graph: dense FFN for one expert (SPMD across 8 cores)."""
    from contextlib import ExitStack

    import concourse.mybir as mybir
    import concourse.tile as tile
    from concourse import bacc

    bf = mybir.dt.bfloat16
    f32 = mybir.dt.float32
    AF = mybir.ActivationFunctionType

    nc = bacc.Bacc(trn_type="TRN2")
    xT = nc.dram_tensor("xT", [D, CAP], bf, kind="ExternalInput").ap()
    w1 = nc.dram_tensor("w1", [D, F_FF], bf, kind="ExternalInput").ap()
    w2 = nc.dram_tensor("w2", [F_FF, D], bf, kind="ExternalInput").ap()
    b1 = nc.dram_tensor("b1", [F_FF], f32, kind="ExternalInput").ap()
    b2 = nc.dram_tensor("b2", [D], f32, kind="ExternalInput").ap()
    out = nc.dram_tensor("out", [D, CAP], f32, kind="ExternalOutput").ap()

    with tile.TileContext(nc) as tc, ExitStack() as ctx:
        wpool = ctx.enter_context(tc.tile_pool(name="weights", bufs=1))
        xpool = ctx.enter_context(tc.tile_pool(name="xin", bufs=2))
        hpool = ctx.enter_context(tc.tile_pool(name="hmid", bufs=1))
        ypool = ctx.enter_context(tc.tile_pool(name="yout", bufs=4))
        # ph + py are distinct tags; 4 bufs each = all 8 PSUM banks
        ppool = ctx.enter_context(tc.tile_pool(name="psum", bufs=4, space="PSUM"))

        # DMA issue order matters: HW-DGE queues are FIFO, so anything
        # emitted before the first token tile's x delays the first matmul
        # by its full transfer time.  Emit x(t=0) first, then w1 in
        # f-column blocks (matmul group fc only depends on its own block),
        # then w2 (first needed ~55us later, in the mm2 phase of t=0).
        # PE warm-up: junk matmuls bridging the first-DMA latency so the
        # HAM clock-gate reaches 2.4 GHz with no re-throttling idle gap
        # (>~3.4us idle drops PE back to 1.2 GHz).
        warm = wpool.tile([P, NT], bf)
        nc.vector.memset(warm, 0.0)
        pwarm = ppool.tile([P, NT], f32, tag="ph")
        for _ in range(18):
            nc.tensor.matmul(pwarm, lhsT=warm[:, :P], rhs=warm, start=True,
                             stop=True)

        xT_r = xT.rearrange("(c p) n -> p c n", p=P)

        def load_x(t, split=1):
            xs = xpool.tile([P, DC, NT], bf, name=f"x_s{t}", tag="xs")
            h = DC // split
            for s in range(split):
                nc.sync.dma_start(
                    out=xs[:, s * h:(s + 1) * h, :],
                    in_=xT_r[:, s * h:(s + 1) * h, t * NT:(t + 1) * NT],
                )
            return xs

        x_cur = load_x(0, split=2)

        # Resident weights: w1 as [p, dc, f] (dc*128+p row of [D,F]),
        # w2 as [p, fc, d] (fc*128+p row of [F,D]).
        FB = 512                      # f-block width for w1 loads
        w1_s = wpool.tile([P, DC, F_FF], bf)
        w1_r = w1.rearrange("(c p) f -> p c f", p=P)
        for fb in range(F_FF // FB):
            # split the first (critical-path) blocks across two queues
            for s in range(2 if fb < 2 else 1):
                h = DC // 2 if fb < 2 else DC
                nc.sync.dma_start(
                    out=w1_s[:, s * h:(s + 1) * h, fb * FB:(fb + 1) * FB],
                    in_=w1_r[:, s * h:(s + 1) * h, fb * FB:(fb + 1) * FB],
                )
        # biases via SWDGE (single queue/sem): the strided partition-dim
        # pattern on HWDGE fans out across queues and blows the per-inst
        # sync-wait budget of the consuming Activation
        b1_s = wpool.tile([P, FC], f32)
        nc.gpsimd.dma_start(out=b1_s, in_=b1.rearrange("(c p) -> p c", p=P))
        b2_s = wpool.tile([P, DC], f32)
        nc.gpsimd.dma_start(out=b2_s, in_=b2.rearrange("(c p) -> p c", p=P))

        w2_s = wpool.tile([P, FC, D], bf)
        w2_r = w2.rearrange("(c p) d -> p c d", p=P)
        CG = 8  # chunks per DMA
        for cg in range(FC // CG):
            nc.sync.dma_start(
                out=w2_s[:, cg * CG:(cg + 1) * CG, :],
                in_=w2_r[:, cg * CG:(cg + 1) * CG, :],
            )

        for t in range(TT):
            x_s = x_cur
            if t + 1 < TT:
                x_cur = load_x(t + 1)
            # h.T tile [f, tok] for this token tile
            h_s = hpool.tile([P, FC, NT], bf)
            for fc in range(FC):
                ph = ppool.tile([P, NT], f32)
                for c in range(DC):
                    nc.tensor.matmul(
                        ph,
                        lhsT=w1_s[:, c, fc * P:(fc + 1) * P],
                        rhs=x_s[:, c, :],
                        start=(c == 0),
                        stop=(c == DC - 1),
                    )
                nc.scalar.activation(
                    h_s[:, fc, :], ph, AF.Gelu, bias=b1_s[:, fc:fc + 1]
                )
            for dc in range(DC):
                # split the kernel's final group so less serial ACT+DMA
                # trails the last matmul
                halves = 2 if (t == TT - 1 and dc == DC - 1) else 1
                w = NT // halves
                for s in range(halves):
                    py = ppool.tile([P, NT], f32, name="py", tag="py")
                    for fc in range(FC):
                        nc.tensor.matmul(
                            py[:, :w],
                            lhsT=w2_s[:, fc, dc * P:(dc + 1) * P],
                            rhs=h_s[:, fc, s * w:(s + 1) * w],
                            start=(fc == 0),
                            stop=(fc == FC - 1),
                        )
                    y_s = ypool.tile([P, NT], f32, name="y_s", tag="ys")
                    nc.scalar.activation(
                        y_s[:, :w], py[:, :w], AF.Identity,
                        bias=b2_s[:, dc:dc + 1]
                    )
                    nc.sync.dma_start(
                        out=out[dc * P:(dc + 1) * P,
                                t * NT + s * w:t * NT + (s + 1) * w],
                        in_=y_s[:, :w],
                    )
    nc.compile()
    return nc


def _build_nc_f32r():
    """float32r variant: fp32 operands, ~tf32 matmul precision, weights
    streamed from HBM every token tile (both stacks can't stay resident
    in fp32).  DMA ~190 MB vs PE ~600us -> at the compute/memory ridge.
    Host pre-packs w1/w2/x into stream-block layouts so every streaming
    DMA is a fully linear copy."""
    from contextlib import ExitStack

    import concourse.mybir as mybir
    import concourse.tile as tile
    from concourse import bacc

    f32 = mybir.dt.float32
    f32r = mybir.dt.float32r
    AF = mybir.ActivationFunctionType

    FB = 512
    NFB = F_FF // FB
    FCB = FB // P  # fc groups per w1 block

    nc = bacc.Bacc(trn_type="TRN2")
    # pre-packed: xT[t,p,c,n], w1[fb,p,c,f], w2[dc,p,fc,d]
    xT = nc.dram_tensor("xT", [TT, P, DC, NT], f32r, kind="ExternalInput").ap()
    w1 = nc.dram_tensor("w1", [NFB, P, DC, FB], f32r, kind="ExternalInput").ap()
    w2 = nc.dram_tensor("w2", [DC, P, FC, P], f32r, kind="ExternalInput").ap()
    b1 = nc.dram_tensor("b1", [F_FF], f32, kind="ExternalInput").ap()
    b2 = nc.dram_tensor("b2", [D], f32, kind="ExternalInput").ap()
    out = nc.dram_tensor("out", [D, CAP], f32, kind="ExternalOutput").ap()

    with tile.TileContext(nc) as tc, ExitStack() as ctx:
        cpool = ctx.enter_context(tc.tile_pool(name="consts", bufs=1))
        xpool = ctx.enter_context(tc.tile_pool(name="xin", bufs=2))
        w1pool = ctx.enter_context(tc.tile_pool(name="w1s", bufs=3))
        w2pool = ctx.enter_context(tc.tile_pool(name="w2s", bufs=3))
        # h head (first 4 fc groups) is double-buffered so the next tile's
        # mm1 pipeline can restart while this tile's mm2 still reads h;
        # the 56 KB tail stays single-buffered (SBUF budget)
        HH = 4
        hhpool = ctx.enter_context(tc.tile_pool(name="hhead", bufs=2))
        hpool = ctx.enter_context(tc.tile_pool(name="hmid", bufs=1))
        ypool = ctx.enter_context(tc.tile_pool(name="yout", bufs=3))
        ppool = ctx.enter_context(tc.tile_pool(name="psum", bufs=4, space="PSUM"))

        warm = cpool.tile([P, NT], mybir.dt.bfloat16)
        nc.vector.memset(warm, 0.0)
        pwarm = ppool.tile([P, NT], f32, tag="ph")
        for _ in range(36):
            nc.tensor.matmul(pwarm, lhsT=warm[:, :P],
                             rhs=warm, start=True, stop=True)

        def load_x(t, split=1):
            xs = xpool.tile([P, DC, NT], f32r, name=f"x_s{t}", tag="xs")
            h = DC // split
            for s in range(split):
                nc.sync.dma_start(
                    out=xs[:, s * h:(s + 1) * h, :],
                    in_=xT[t, :, s * h:(s + 1) * h, :],
                )
            return xs

        def load_w1(fb, split=1):
            wb = w1pool.tile([P, DC, FB], f32r, name=f"w1b{fb}", tag="w1b")
            h = DC // split
            for s in range(split):
                nc.sync.dma_start(
                    out=wb[:, s * h:(s + 1) * h, :],
                    in_=w1[fb, :, s * h:(s + 1) * h, :],
                )
            return wb

        def load_w2(dc):
            wb = w2pool.tile([P, FC, P], f32r, name=f"w2b{dc}", tag="w2b")
            nc.sync.dma_start(out=wb, in_=w2[dc])
            return wb

        # critical startup prefix: x(0) and the first two w1 blocks
        x_cur = load_x(0, split=2)
        w1_q = [load_w1(0, split=2), load_w1(1, split=2)]

        b1_s = cpool.tile([P, FC], f32)
        nc.gpsimd.dma_start(out=b1_s, in_=b1.rearrange("(c p) -> p c", p=P))
        b2_s = cpool.tile([P, DC], f32)
        nc.gpsimd.dma_start(out=b2_s, in_=b2.rearrange("(c p) -> p c", p=P))

        def next_w1(t, fb):
            """Block to prefetch while (t, fb) is being consumed, keeping
            two blocks in flight."""
            nfb = fb + 2
            nt = t
            if nfb >= NFB:
                nfb -= NFB
                nt += 1
            return None if nt >= TT else nfb

        for t in range(TT):
            x_s = x_cur
            if t + 1 < TT:
                x_cur = load_x(t + 1)
            hh_s = hhpool.tile([P, HH, NT], f32r, name="hh_s", tag="hh")
            h_s = hpool.tile([P, FC - HH, NT], f32r)

            def h_at(fc):
                return hh_s[:, fc, :] if fc < HH else h_s[:, fc - HH, :]

            for fb in range(NFB):
                wb = w1_q.pop(0)
                pf = next_w1(t, fb)
                if pf is not None:
                    w1_q.append(load_w1(pf))
                # interleave group pairs (two PSUM banks in flight) so each
                # group's start latency hides behind the other's matmuls
                for fp in range(FCB // 2):
                    fcl0, fcl1 = 2 * fp, 2 * fp + 1
                    fc0, fc1 = fb * FCB + fcl0, fb * FCB + fcl1
                    ph0 = ppool.tile([P, NT], f32, name="ph0", tag="ph")
                    ph1 = ppool.tile([P, NT], f32, name="ph1", tag="ph")
                    for c in range(DC):
                        nc.tensor.matmul(
                            ph0,
                            lhsT=wb[:, c, fcl0 * P:(fcl0 + 1) * P],
                            rhs=x_s[:, c, :],
                            start=(c == 0),
                            stop=(c == DC - 1),
                        )
                        nc.tensor.matmul(
                            ph1,
                            lhsT=wb[:, c, fcl1 * P:(fcl1 + 1) * P],
                            rhs=x_s[:, c, :],
                            start=(c == 0),
                            stop=(c == DC - 1),
                        )
                    nc.scalar.activation(
                        h_at(fc0), ph0, AF.Gelu, bias=b1_s[:, fc0:fc0 + 1]
                    )
                    nc.scalar.activation(
                        h_at(fc1), ph1, AF.Gelu, bias=b1_s[:, fc1:fc1 + 1]
                    )
            w2_q = [load_w2(0), load_w2(1)]
            for dc in range(DC):
                w2b = w2_q.pop(0)
                if dc + 2 < DC:
                    w2_q.append(load_w2(dc + 2))
                halves = 2 if (t == TT - 1 and dc == DC - 1) else 1
                w = NT // halves
                for s in range(halves):
                    py = ppool.tile([P, NT], f32, name="py", tag="py")
                    for fc in range(FC):
                        nc.tensor.matmul(
                            py[:, :w],
                            lhsT=w2b[:, fc, :],
                            rhs=h_at(fc)[:, s * w:(s + 1) * w],
                            start=(fc == 0),
                            stop=(fc == FC - 1),
                        )
                    y_s = ypool.tile([P, NT], f32, name="y_s", tag="ys")
                    nc.scalar.activation(
                        y_s[:, :w], py[:, :w], AF.Identity,
                        bias=b2_s[:, dc:dc + 1]
                    )
                    nc.sync.dma_start(
                        out=out[dc * P:(dc + 1) * P,
                                t * NT + s * w:t * NT + (s + 1) * w],
                        in_=y_s[:, :w],
                    )
    nc.compile()
    return nc


def _route(x, w_router, b_router):
    """Replicates reference routing (softmax -> top-2 -> capacity) in f64.

    Returns per-expert (token_ids, slot_positions, gate_values)."""
    xf = x.reshape(N, D).astype(np.float64)
    logits = xf @ w_router.astype(np.float64) + b_router.astype(np.float64)
    logits -= logits.max(axis=-1, keepdims=True)
    p = np.exp(logits)
    gates = p / p.sum(axis=-1, keepdims=True)
    # top-2, ties to the lower index (matches lax.top_k)
    order = np.argsort(-gates, axis=1, kind="stable")[:, :TOP_K]
    topv = np.take_along_axis(gates, order, axis=1)
    e_flat = order.reshape(-1)
    g_flat = topv.reshape(-1).astype(np.float32)
    tok = np.repeat(np.arange(N), TOP_K)
    pos = np.empty(N * TOP_K, np.int64)
    for e in range(E):
        m_e = e_flat == e
        pos[m_e] = np.arange(int(m_e.sum()))
    keep = pos < CAP
    per_expert = []
    for e in range(E):
        sel = (e_flat == e) & keep
        per_expert.append((tok[sel], pos[sel], g_flat[sel]))
    return per_expert


def _run_device(in_maps, trace=False):
    from concourse.bass_utils import run_bass_kernel_spmd

    if "nc" not in _NC_CACHE:
        _NC_CACHE["nc"] = _build_nc() if MODE == "bf16" else _build_nc_f32r()
    return run_bass_kernel_spmd(
        _NC_CACHE["nc"], in_maps, core_ids=list(range(NCORES)), trace=trace
    )


def _kernel_impl(inputs, trace=False):
    x = np.asarray(inputs["x"], dtype=np.float32)
    w_router = np.asarray(inputs["w_router"], dtype=np.float32)
    b_router = np.asarray(inputs["b_router"], dtype=np.float32)
    w1 = np.asarray(inputs["w1"], dtype=np.float32)
    b1 = np.ascontiguousarray(np.asarray(inputs["b1"], dtype=np.float32))
    w2 = np.asarray(inputs["w2"], dtype=np.float32)
    b2 = np.ascontiguousarray(np.asarray(inputs["b2"], dtype=np.float32))

    per_expert = _route(x, w_router, b_router)
    xf = x.reshape(N, D)

    if MODE == "bf16":
        w1d = w1.astype(BF16)
        w2d = w2.astype(BF16)
    in_maps = []
    for e in range(E):
        tk, ps, _ = per_expert[e]
        buf = np.zeros((CAP, D), np.float32)
        buf[ps] = xf[tk]
        bufT = np.ascontiguousarray(buf.T)          # [D, CAP]
        if MODE == "bf16":
            in_maps.append({
                "xT": bufT.astype(BF16),
                "w1": np.ascontiguousarray(w1d[e]),
                "w2": np.ascontiguousarray(w2d[e]),
                "b1": b1[e],
                "b2": b2[e],
            })
        else:
            # stream-block layouts: x[t,p,c,n], w1[fb,p,c,f], w2[dc,p,fc,d]
            xp = np.ascontiguousarray(
                bufT.reshape(DC, P, TT, NT).transpose(2, 1, 0, 3))
            w1p = np.ascontiguousarray(
                w1[e].reshape(DC, P, 8, 512).transpose(2, 1, 0, 3))
            w2p = np.ascontiguousarray(
                w2[e].reshape(FC, P, DC, P).transpose(2, 1, 0, 3))
            in_maps.append({
                "xT": xp, "w1": w1p, "w2": w2p, "b1": b1[e], "b2": b2[e],
            })

    res = _run_device(in_maps, trace=trace)

    y = np.zeros((N, D), np.float32)
    ws = np.zeros((N,), np.float32)
    for e in range(E):
        tk, ps, gv = per_expert[e]
        outT = res.results[e]["out"]          # [D, CAP] f32
        vals = (outT[:, ps] * gv[None, :]).T  # [n_e, D]
        y[tk] += vals                         # tk unique within one expert
        ws[tk] += gv
    y = np.where((ws > 0.0)[:, None], y / np.maximum(ws, 1e-6)[:, None], y)
    return y.reshape(B, T, D).astype(np.float32), res


def kernel(**inputs):
    y, _ = _kernel_impl(inputs, trace=False)
    return y
